# revision 28
# baseline (speedup 1.0000x reference)
"""Trainium2 Bass kernel for BatchEnsemble encoder-decoder multihead attention.

Problem (hardcoded shapes): Tq=Tk=1024, B=8, H=1024, heads=16, hd=64.

Sharding: pure data parallelism — batch B=8 across the 8 NeuronCores, one
batch element per core. No collectives.

All matmuls run in bf16 (1 PE cycle/row, same as fp32r, but no >=256-row
requirement), with fp32 PSUM accumulation. BatchEnsemble rank-1 factors and
the 1/sqrt(hd) scale are folded into per-core weight matrices on the host.

Per-core math (batch b):
    V   = Xk^T.T @ Wv''^T            [Tk, H]   + interleaved ones columns
    K^T = Wk''^T.T @ Xk^T            [H, Tk]   (features on partitions)
    Q^T = Wq''^T.T @ Xq^T            [H, Tq]
    per pair j (heads 2j @ rows 0-63, 2j+1 @ 64-127), query half qb:
      S^T tile = K_g^T.T @ Q_g^T     [128 keys, 2x512 queries] per key chunk i
      P~ = exp(S^T)                   (scalar engine; scores bounded)
      transposed ctx: ct[q, 65] += P~chunk.T @ [V_g | 1]   (65-row moving ops,
           stationary = probs chunk; denominator rides along as column 64)
      normalize: ctx[q, d] = ct[q, d] * (1/ct[q, 64])  (per-partition scalar,
           no cross-partition broadcast needed)
    ctx^T via XBAR DMA transpose (16x128 tiles, off the PE entirely)
    out = ctx^T.T @ Wo^T             [Tq, H]

Schedule: the scalar engine's ~133us of Exp work is the secondary pacer (PE
~191us is primary), so everything is interleaved at scores-TILE granularity:
after V-proj + K/Q(j0,j1) head waves, 16 scores blocks stream at Exp pace with
(a) the previous block's ctx accumulation rounds and (b) the remaining K/Q
projection chains + first-half output projection, chopped into <=0.9us
generator steps, pumped into the per-tile slack. The 2-deep scores-psum ring
locks PE<->Act into per-tile lockstep, which is why filler chunks must stay
under ~2 exp periods. Block order: j-major for j0,j1 (both query halves),
then qb=0 for j2-7, then qb=1 (out-proj first half as filler, queued after
the qb0 ctx transposes). Tail: last pair's ctx rounds on the freed scores
banks, XBAR transposes interleaved, second-half out-proj, bf16 writeback
(host upcasts). The wv SBUF pool doubles as the exp-tile ring once V-proj
retires (lifetime-disjoint reuse).
"""

import numpy as np
import ml_dtypes

import concourse.bass as bass
import concourse.tile as tile
import concourse.mybir as mybir
from concourse import bacc
from concourse.bass_utils import run_bass_kernel_spmd

F32 = mybir.dt.float32
BF16 = mybir.dt.bfloat16
AF = mybir.ActivationFunctionType

T = 1024        # Tq = Tk
H = 1024
B = 8
HEADS = 16
HD = 64
NT = T // 128   # 8 x 128-chunks
NB = T // 512   # 2 x 512-blocks
PAIRS = HEADS // 2

_cache = {}
_last_in_maps = None


def _build(with_bq, with_bk, with_bv):
    nc = bacc.Bacc("TRN2", target_bir_lowering=False, debug=False)

    xqt_d = nc.dram_tensor("xqt", [H, T], BF16, kind="ExternalInput")
    xkt_d = nc.dram_tensor("xkt", [H, T], BF16, kind="ExternalInput")
    wqt_d = nc.dram_tensor("wqt", [H, H], BF16, kind="ExternalInput")
    wkt_d = nc.dram_tensor("wkt", [H, H], BF16, kind="ExternalInput")
    wvt_d = nc.dram_tensor("wvt", [H, H], BF16, kind="ExternalInput")
    wot_d = nc.dram_tensor("wot", [H, H], BF16, kind="ExternalInput")
    bq_d = nc.dram_tensor("bq", [H], F32, kind="ExternalInput") if with_bq else None
    bk_d = nc.dram_tensor("bk", [H], F32, kind="ExternalInput") if with_bk else None
    bv_d = nc.dram_tensor("bv", [H], BF16, kind="ExternalInput") if with_bv else None
    out_d = nc.dram_tensor("out", [T, H], BF16, kind="ExternalOutput")

    with tile.TileContext(nc) as tc:
        with tc.tile_pool(name="pxk", bufs=8) as pxk, \
             tc.tile_pool(name="pwk", bufs=8) as pwk, \
             tc.tile_pool(name="pbig", bufs=21) as pbig, \
             tc.tile_pool(name="pxq", bufs=8) as pxq, \
             tc.tile_pool(name="pwq", bufs=8) as pwq, \
             tc.tile_pool(name="pwo", bufs=8) as pwo, \
             tc.tile_pool(name="pqt", bufs=8) as pqt, \
             tc.tile_pool(name="pkt", bufs=8) as pkt, \
             tc.tile_pool(name="pv", bufs=8) as pv, \
             tc.tile_pool(name="pcq", bufs=8) as pcq, \
             tc.tile_pool(name="pctT", bufs=2) as pctT, \
             tc.tile_pool(name="prc", bufs=4) as prc, \
             tc.tile_pool(name="po", bufs=4) as po, \
             tc.tile_pool(name="pbias", bufs=4) as pbias, \
             tc.tile_pool(name="psS", bufs=2, space="PSUM") as psS, \
             tc.tile_pool(name="psP", bufs=4, space="PSUM") as psP:

            # ---- Act table pre-warm: dummy exp while DMAs stream ----
            warm = pbias.tile([1, 8], F32, tag="bias", name="warm")
            nc.vector.memset(warm, 0.0)
            nc.scalar.activation(out=warm, in_=warm, func=AF.Exp)

            # ---- input DMAs, in consumption order: xk+wv, wk, xq+wq, wo ----
            xkt, wv = [], []
            for h in range(NT):
                t_ = pxk.tile([128, T], BF16, tag="pxk", name=f"xkt{h}")
                nc.sync.dma_start(out=t_, in_=xkt_d[h * 128:(h + 1) * 128, :])
                xkt.append(t_)
                w_ = pbig.tile([128, H], BF16, tag="big", name=f"wv{h}")
                nc.sync.dma_start(out=w_, in_=wvt_d[h * 128:(h + 1) * 128, :])
                wv.append(w_)
            wk = []
            for h in range(NT):
                w_ = pwk.tile([128, H], BF16, tag="pwk", name=f"wk{h}")
                nc.sync.dma_start(out=w_, in_=wkt_d[h * 128:(h + 1) * 128, :])
                wk.append(w_)
            xqt, wq = [], []
            for h in range(NT):
                t_ = pxq.tile([128, T], BF16, tag="pxq", name=f"xqt{h}")
                nc.sync.dma_start(out=t_, in_=xqt_d[h * 128:(h + 1) * 128, :])
                xqt.append(t_)
                w_ = pwq.tile([128, H], BF16, tag="pwq", name=f"wq{h}")
                nc.sync.dma_start(out=w_, in_=wqt_d[h * 128:(h + 1) * 128, :])
                wq.append(w_)
            wo = []
            for h in range(NT):
                w_ = pwo.tile([128, H], BF16, tag="pwo", name=f"wo{h}")
                nc.sync.dma_start(out=w_, in_=wot_d[h * 128:(h + 1) * 128, :])
                wo.append(w_)

            if with_bq:
                bq_t = pbias.tile([128, NT], F32, tag="bias", name="bq_t")
                nc.sync.dma_start(out=bq_t, in_=bq_d.rearrange("(j p) -> p j", p=128))
            if with_bk:
                bk_t = pbias.tile([128, NT], F32, tag="bias", name="bk_t")
                nc.sync.dma_start(out=bk_t, in_=bk_d.rearrange("(j p) -> p j", p=128))
            if with_bv:
                bv_t = pbias.tile([1, H], BF16, tag="bias", name="bv_t")
                nc.sync.dma_start(out=bv_t, in_=bv_d.rearrange("h -> 1 h"))
                ones1 = pbias.tile([1, 128], BF16, tag="bias", name="ones1")
                nc.vector.memset(ones1, 1.0)

            # ---- persistent result tiles ----
            kt = [pkt.tile([128, T], BF16, tag="pkt", name=f"kt{j}")
                  for j in range(NT)]
            qt = [pqt.tile([128, T], BF16, tag="pqt", name=f"qt{j}")
                  for j in range(NT)]
            vbuf = []
            for i in range(NT):
                vb = pv.tile([128, HEADS * 65], BF16, tag="pv", name=f"vb{i}")
                nc.vector.memset(
                    vb.rearrange("p (g c) -> p g c", c=65)[:, :, 64:65], 1.0)
                vbuf.append(vb)
            ctxq = [pcq.tile([128, H], BF16, tag="pcq", name=f"cq{c}")
                    for c in range(NT)]
            # one ctx^T tile per qb half to avoid false deps between halves:
            # ctxT[qb][p, j*512 + (c-4*qb)*128 + q] = ctx[query c*128+q, feat j*128+p]
            ctxT = [pctT.tile([128, NT * 512], BF16, tag="pctT", name=f"ctT{b_}")
                    for b_ in range(2)]

            # ---- V projection: two 8-chain waves; V[t, f] into vbuf strided ----
            def v_wave(ilist):
                pS = [psS.tile([128, 1024], F32, tag="s", name=f"vw{ilist[0]}{c}")
                      for c in range(2)]
                pP = [psP.tile([128, 512], F32, tag="p", name=f"vwp{ilist[0]}{c}")
                      for c in range(4)]
                slots = [pS[0][:, 0:512], pS[0][:, 512:1024],
                         pS[1][:, 0:512], pS[1][:, 512:1024]] + pP
                chains = [(i, ob) for i in ilist for ob in range(NB)]
                for h in range(NT):
                    last = (h == NT - 1) and not with_bv
                    for sl, (i, ob) in zip(slots, chains):
                        nc.tensor.matmul(
                            sl, xkt[h][:, i * 128:(i + 1) * 128],
                            wv[h][:, ob * 512:(ob + 1) * 512],
                            start=(h == 0), stop=last)
                if with_bv:
                    for sl, (i, ob) in zip(slots, chains):
                        nc.tensor.matmul(
                            sl, ones1, bv_t[:, ob * 512:(ob + 1) * 512],
                            start=False, stop=True)
                for sl, (i, ob) in list(zip(slots, chains))[4:] + \
                        list(zip(slots, chains))[:4]:
                    dst = vbuf[i][:, ob * 8 * 65:(ob + 1) * 8 * 65] \
                        .rearrange("p (g c) -> p g c", c=65)[:, :, 0:64]
                    nc.vector.tensor_copy(
                        out=dst, in_=sl.rearrange("p (g d) -> p g d", d=64))

            v_wave([0, 1, 2, 3])
            v_wave([4, 5, 6, 7])

            # ---- K projection: 8-chain wave j0-3 now; j4-7 as pass-0 filler ----
            def k_wave8(jlist):
                pS = [psS.tile([128, 1024], F32, tag="s", name=f"kw{jlist[0]}{c}")
                      for c in range(2)]
                pP = [psP.tile([128, 512], F32, tag="p", name=f"kwp{jlist[0]}{c}")
                      for c in range(4)]
                slots = [pS[0][:, 0:512], pS[0][:, 512:1024],
                         pS[1][:, 0:512], pS[1][:, 512:1024]] + pP
                chains = [(j, tb) for j in jlist for tb in range(NB)]
                for h in range(NT):
                    for sl, (j, tb) in zip(slots, chains):
                        nc.tensor.matmul(
                            sl, wk[h][:, j * 128:(j + 1) * 128],
                            xkt[h][:, tb * 512:(tb + 1) * 512],
                            start=(h == 0), stop=(h == NT - 1))
                for sl, (j, tb) in zip(slots, chains):
                    d = kt[j][:, tb * 512:(tb + 1) * 512]
                    if with_bk:
                        nc.vector.tensor_scalar_add(d, sl, bk_t[:, j:j + 1])
                    else:
                        nc.vector.tensor_copy(out=d, in_=sl)

            def k_wave4(jlist):
                # psP-only: safe to interleave with the scores pipeline,
                # which owns psS
                pP = [psP.tile([128, 512], F32, tag="p", name=f"kf{jlist[0]}{c}")
                      for c in range(2 * len(jlist))]
                chains = [(j, tb) for j in jlist for tb in range(NB)]
                for h in range(NT):
                    for sl, (j, tb) in zip(pP, chains):
                        nc.tensor.matmul(
                            sl, wk[h][:, j * 128:(j + 1) * 128],
                            xkt[h][:, tb * 512:(tb + 1) * 512],
                            start=(h == 0), stop=(h == NT - 1))
                for sl, (j, tb) in zip(pP, chains):
                    d = kt[j][:, tb * 512:(tb + 1) * 512]
                    if with_bk:
                        nc.vector.tensor_scalar_add(d, sl, bk_t[:, j:j + 1])
                    else:
                        nc.vector.tensor_copy(out=d, in_=sl)


            # ---- Q projection: psP-only waves ----
            def q_wave(jlist):
                pP = [psP.tile([128, 512], F32, tag="p", name=f"qw{jlist[0]}{c}")
                      for c in range(2 * len(jlist))]
                chains = [(j, tb) for j in jlist for tb in range(NB)]
                for h in range(NT):
                    for sl, (j, tb) in zip(pP, chains):
                        nc.tensor.matmul(
                            sl, wq[h][:, j * 128:(j + 1) * 128],
                            xqt[h][:, tb * 512:(tb + 1) * 512],
                            start=(h == 0), stop=(h == NT - 1))
                for sl, (j, tb) in zip(pP, chains):
                    d = qt[j][:, tb * 512:(tb + 1) * 512]
                    if with_bq:
                        nc.vector.tensor_scalar_add(d, sl, bq_t[:, j:j + 1])
                    else:
                        nc.vector.tensor_copy(out=d, in_=sl)

            k_wave4([0, 1])
            q_wave([0, 1])

            # ---- attention building blocks ----
            ex_tiles = {}

            def sc_tile(j, qb, i):
                ss = psS.tile([128, 1024], F32, tag="s", name=f"ss{j}{qb}{i}")
                ex = pbig.tile([128, 1024], BF16, tag="big", name=f"ex{j}{qb}{i}")
                for p in range(2):
                    r0 = p * 64
                    nc.tensor.matmul(
                        ss[:, p * 512:(p + 1) * 512],
                        kt[j][r0:r0 + 64, i * 128:(i + 1) * 128],
                        qt[j][r0:r0 + 64, qb * 512:(qb + 1) * 512],
                        start=True, stop=True)
                nc.scalar.activation(out=ex, in_=ss, func=AF.Exp)
                ex_tiles[(j, qb, i)] = ex

            def ctx_round(j, qb, r, ch=None, norm_on_act=False):
                # round r = (half, p): 2 chains (ql in {0,1}), each owning a
                # full psum bank; reads all 8 exp tiles of block (j, qb)
                half, p = r // 2, r % 2
                g = 2 * j + p
                if ch is None:
                    ch = [psP.tile([128, 512], F32, tag="p",
                                   name=f"ct{j}{qb}{r}{ql}")
                          for ql in range(2)]
                for i in range(NT):
                    ex = ex_tiles[(j, qb, i)]
                    for ql in range(2):
                        qq = half * 2 + ql
                        nc.tensor.matmul(
                            ch[ql][:, 0:65],
                            ex[:, p * 512 + qq * 128:p * 512 + (qq + 1) * 128],
                            vbuf[i][:, g * 65:(g + 1) * 65],
                            start=(i == 0), stop=(i == NT - 1))
                if r == 3:
                    for i in range(NT):
                        ex_tiles.pop((j, qb, i), None)
                # normalize: per-partition scalar multiply by 1/denom
                for ql in range(2):
                    qq = half * 2 + ql
                    c = qb * 4 + qq
                    rc = prc.tile([128, 1], F32, tag="rc",
                                  name=f"rc{j}{qb}{qq}{p}")
                    nc.vector.reciprocal(out=rc, in_=ch[ql][:, 64:65])
                    dst = ctxq[c][:, j * 128 + p * 64:j * 128 + (p + 1) * 64]
                    if norm_on_act:
                        nc.scalar.activation(out=dst, in_=ch[ql][:, 0:64],
                                             func=AF.Copy, scale=rc[:, 0:1])
                    else:
                        nc.vector.tensor_scalar_mul(dst, ch[ql][:, 0:64],
                                                    rc[:, 0:1])

            # ---- filler generators: small PE steps pumped between scores
            # tiles so the scalar engine (the pacer) never starves ----
            def gen_proj_chain(wgt, win, dst, j, tb, bias_t):
                # one 512-wide chain: 8 chained matmuls + drain copy
                sl = psP.tile([128, 512], F32, tag="p", name=f"fc{j}{tb}")
                for h in range(NT):
                    nc.tensor.matmul(
                        sl, wgt[h][:, j * 128:(j + 1) * 128],
                        win[h][:, tb * 512:(tb + 1) * 512],
                        start=(h == 0), stop=(h == NT - 1))
                    if h % 2 == 1:
                        yield 430
                d = dst[j][:, tb * 512:(tb + 1) * 512]
                if bias_t is not None:
                    nc.vector.tensor_scalar_add(d, sl, bias_t[:, j:j + 1])
                else:
                    nc.vector.tensor_copy(out=d, in_=sl)
                yield 0

            def gen_op_chain(tt, ob):
                qb, cc = tt // 4, tt % 4
                sl = psP.tile([128, 512], F32, tag="p", name=f"fo{tt}{ob}")
                for j in range(NT):
                    nc.tensor.matmul(
                        sl,
                        ctxT[qb][:, j * 512 + cc * 128:j * 512 + (cc + 1) * 128],
                        wo[j][:, ob * 512:(ob + 1) * 512],
                        start=(j == 0), stop=(j == NT - 1))
                    if j % 2 == 1:
                        yield 430
                o_ = po.tile([128, 512], BF16, tag="po", name=f"fot{tt}{ob}")
                nc.vector.tensor_copy(out=o_, in_=sl)
                nc.sync.dma_start(
                    out=out_d[tt * 128:(tt + 1) * 128, ob * 512:(ob + 1) * 512],
                    in_=o_)
                yield 0

            filler_q = []

            def pump(budget):
                while budget > 0 and filler_q:
                    name, g = filler_q[0]
                    try:
                        budget -= next(g)
                    except StopIteration:
                        filler_q.pop(0)

            def drain_through(name):
                # finish queued fillers from the front until no entry with
                # `name` remains
                while any(nm == name for nm, _ in filler_q):
                    nm, g = filler_q[0]
                    for _ in g:
                        pass
                    filler_q.pop(0)

            def block(j, qb, prev, budget=840):
                # one scores block with the previous block's ctx rounds and
                # filler steps interleaved at tile granularity
                for i in range(NT):
                    sc_tile(j, qb, i)
                    if prev is not None and i in (2, 4, 5, 7):
                        ctx_round(prev[0], prev[1], (2, 4, 5, 7).index(i))
                    else:
                        pump(budget)

            def transpose_c(qb, c):
                out_ap = ctxT[qb].rearrange("p (j q) -> p j q", q=512)[
                    :, :, (c - qb * 4) * 128:(c - qb * 4 + 1) * 128]
                nc.sync.dma_start_transpose(out_ap, ctxq[c])

            def transpose_half(qb):
                for c in range(qb * 4, qb * 4 + 4):
                    transpose_c(qb, c)

            def out_proj(tt):
                qb, cc = tt // 4, tt % 4
                ps2 = [psP.tile([128, 512], F32, tag="p", name=f"op{tt}{ob}")
                       for ob in range(NB)]
                for j in range(NT):
                    for ob in range(NB):
                        nc.tensor.matmul(
                            ps2[ob],
                            ctxT[qb][:, j * 512 + cc * 128:j * 512 + (cc + 1) * 128],
                            wo[j][:, ob * 512:(ob + 1) * 512],
                            start=(j == 0), stop=(j == NT - 1))
                for ob in range(NB):
                    o_ = po.tile([128, 512], BF16, tag="po", name=f"ot{tt}{ob}")
                    if ob == 0:
                        nc.scalar.copy(out=o_, in_=ps2[ob])
                    else:
                        nc.vector.tensor_copy(out=o_, in_=ps2[ob])
                    nc.sync.dma_start(
                        out=out_d[tt * 128:(tt + 1) * 128, ob * 512:(ob + 1) * 512],
                        in_=o_)

            # ---- queue the remaining projections as tile-granular fillers ----
            for j in range(2, NT):
                for tb in range(NB):
                    filler_q.append((f"k{j}", gen_proj_chain(
                        wk, xkt, kt, j, tb, bk_t if with_bk else None)))
                for tb in range(NB):
                    filler_q.append((f"q{j}", gen_proj_chain(
                        wq, xqt, qt, j, tb, bq_t if with_bq else None)))

            # ---- pass A: qb=0 (plus j0,j1 @ qb=1 early, j-major) ----
            seqA = [(0, 0), (0, 1), (1, 0), (1, 1),
                    (2, 0), (3, 0), (4, 0), (5, 0), (6, 0), (7, 0)]
            prev = None
            for (j, qb) in seqA:
                if qb == 0 and j >= 1:
                    drain_through(f"k{j}")
                    drain_through(f"q{j}")
                block(j, qb, prev)
                prev = (j, qb)

            # ---- pass B: remaining qb=1 blocks; out-proj tt0..3 fillers ----
            for (j, qb) in [(2, 1), (3, 1), (4, 1), (5, 1), (6, 1), (7, 1)]:
                block(j, qb, prev, budget=640)
                prev = (j, qb)
                if (j, qb) == (2, 1):
                    # prev ctx here was (7,0): qb0 ctx complete -> transpose,
                    # then out-proj for the first query half joins the queue
                    transpose_half(0)
                    for tt in range(4):
                        for ob in range(NB):
                            filler_q.append(("op", gen_op_chain(tt, ob)))
            while filler_q:
                drain_through(filler_q[0][0])
            tails = [psS.tile([128, 1024], F32, tag="s", name=f"tl{k}")
                     for k in range(2)]
            ctx_round(7, 1, 0, ch=[tails[0][:, 0:512], tails[0][:, 512:1024]],
                      norm_on_act=False)
            ctx_round(7, 1, 1, ch=[tails[1][:, 0:512], tails[1][:, 512:1024]],
                      norm_on_act=False)
            transpose_c(1, 4)
            transpose_c(1, 5)
            tails2 = [psS.tile([128, 1024], F32, tag="s", name=f"tl2{k}")
                      for k in range(2)]
            ctx_round(7, 1, 2, ch=[tails2[0][:, 0:512], tails2[0][:, 512:1024]],
                      norm_on_act=False)
            ctx_round(7, 1, 3, ch=[tails2[1][:, 0:512], tails2[1][:, 512:1024]],
                      norm_on_act=False)
            transpose_c(1, 6)
            transpose_c(1, 7)
            for tt in range(4, 8):
                out_proj(tt)

    nc.finalize()
    return nc


def kernel(inputs_q, inputs_kv, w_q, b_q, w_kv, b_kv, w_o, b_o,
           r_q, s_q, r_kv, s_kv, heads):
    inputs_q = np.asarray(inputs_q, np.float32)
    inputs_kv = np.asarray(inputs_kv, np.float32)
    w_q = np.asarray(w_q, np.float32)
    b_q = np.asarray(b_q, np.float32)
    w_kv = np.asarray(w_kv, np.float32)
    b_kv = np.asarray(b_kv, np.float32)
    w_o = np.asarray(w_o, np.float32)
    b_o = np.asarray(b_o, np.float32)
    r_q = np.asarray(r_q, np.float32)
    s_q = np.asarray(s_q, np.float32)
    r_kv = np.asarray(r_kv, np.float32)
    s_kv = np.asarray(s_kv, np.float32)
    heads = int(heads)
    assert heads == HEADS and inputs_q.shape == (T, B, H)

    scale = np.float32((H // heads) ** -0.5)

    # split w_kv / b_kv / s_kv into K and V parts (2H axis = heads x {k,v} x hd)
    w_kv_r = w_kv.reshape(HEADS, 2, HD, H)
    k_w = w_kv_r[:, 0].reshape(H, H)
    v_w = w_kv_r[:, 1].reshape(H, H)
    b_kv_r = b_kv.reshape(HEADS, 2, HD)
    bk = np.ascontiguousarray(b_kv_r[:, 0].reshape(H))
    bv = np.ascontiguousarray(b_kv_r[:, 1].reshape(H))
    s_kv_r = s_kv.reshape(B, HEADS, 2, HD)
    s_k = s_kv_r[:, :, 0].reshape(B, H)
    s_v = s_kv_r[:, :, 1].reshape(B, H)

    with_bq = bool(np.any(b_q))
    with_bk = bool(np.any(bk))
    with_bv = bool(np.any(bv))
    key = (with_bq, with_bk, with_bv)
    if key not in _cache:
        _cache[key] = _build(*key)
    nc = _cache[key]

    bf = ml_dtypes.bfloat16
    wot = np.ascontiguousarray(w_o.T).astype(bf)
    in_maps = []
    for b in range(B):
        m = {
            "xqt": np.ascontiguousarray(inputs_q[:, b, :].T).astype(bf),
            "xkt": np.ascontiguousarray(inputs_kv[:, b, :].T).astype(bf),
            # W''[o,h] = s[o]*W[o,h]*r[h]; lhsT wants [h, o] = W''.T
            "wqt": np.ascontiguousarray(
                (w_q * (s_q[b] * scale)[:, None] * r_q[b][None, :]).T).astype(bf),
            "wkt": np.ascontiguousarray(
                (k_w * s_k[b][:, None] * r_kv[b][None, :]).T).astype(bf),
            "wvt": np.ascontiguousarray(
                (v_w * s_v[b][:, None] * r_kv[b][None, :]).T).astype(bf),
            "wot": wot,
        }
        if with_bq:
            m["bq"] = b_q * scale
        if with_bk:
            m["bk"] = bk
        if with_bv:
            m["bv"] = bv.astype(bf)
        in_maps.append(m)

    global _last_in_maps
    _last_in_maps = in_maps
    res = run_bass_kernel_spmd(nc, in_maps, list(range(B)))
    out = np.empty((T, B, H), np.float32)
    for b in range(B):
        out[:, b, :] = np.asarray(res.results[b]["out"], np.float32)
    out += b_o
    return out
